# revision 56
# baseline (speedup 1.0000x reference)
"""Trainium2 Bass kernel for BitwiseTasNetBlock.

Data-parallel over batch: 8 cores x 2 batch items. All matmul operands and
activations are bf16 (weights are binarized +-1 so bf16 is exact for them;
activation rounding is ~0.3% which is far inside the 2e-2 gate); PSUM and
all BN statistics stay fp32.

Per layer:
  A: w1 1x1 conv (bf16 matmuls; layer 0 consumes the DMA-staged f32 input
     directly as f32r, skipping conversion copies) + PReLU (ACT Prelu,
     fused psum->sbuf) + bn_stats (DVE)
  sync1: per-channel (sum, sumsq) AllGather (cheaper than AllReduce on the
     CC path) + local DVE reduce -> BN1 scale/shift; 8 chunks of dconv
     run-ahead (6 psab + 2 borrowed psc banks) keep the PE busy meanwhile
  B: depthwise dilated conv as diagonal matmuls on PE (3 taps, psum accum),
     BN1 applied via ACT scale/bias on psum->sbuf; PReLU2 fused; bn_stats2.
     The causal pads are rewritten with u = -t1/s1 after sync1, which makes
     every pad tap contribute exactly the missing bias term - one uniform
     apply, no boundary-strip fixups.
  sync2: AllGather -> BN2 folded into w2 weights (per-partition scale)
  C: w2 1x1 conv + bias, drains alternate DVE/ACT; last layer adds the
     residual from an SBUF-resident bf16 copy of x (made on the idle Pool
     engine; no HBM re-load) and streams the output DMAs.
Y1 activations live in one persistent SBUF buffer per channel-tile; P2
(the dconv output) is written in-place into retired Y1 chunk slots.
"""
import sys

sys.path.insert(0, "/opt/trn_rl_repo")
import numpy as np

L, CB, D, KTAP = 4, 256, 512, 3
B, T = 16, 4096
EPS = 1e-5
NCORES = 8
BLOC = B // NCORES  # 2 batch items per core
CHUNK = 512
NCT = T // CHUNK  # 8 time chunks per batch item
NCHUNKS = BLOC * NCT  # 16 chunk-iterations per phase
PAD = 16
DOFF = PAD  # Y1 data offset inside ybuf
YCOLS = PAD + T
NLOC = float(BLOC * T)  # per-core elements per channel
NGLOB = float(B * T)  # global elements per channel

# Chunks are processed in REVERSE time order in every phase. That makes
# Y1[c] dead right after dconv(c) (window(c-1) ends where Y1[c] starts),
# so P2[c] overwrites Y1[c]'s own slot exactly - no extra SBUF slot.
REV = list(range(NCT - 1, -1, -1))

# packed per-channel vector indices
# K_x = be1*ws_x + bd and W_x = g1*ws_x are host-folded so the post-sync1
# bias variants are b2x = K_x - (mean*rstd)*W_x (2 ops each).
(V_B1, V_BD, V_G1, V_BE1, V_G2, V_BE2, V_KA, V_K12, V_K2,
 V_WD0, V_WD1, V_WD2, V_WA, V_W12, V_W2) = range(15)
NVEC = 15

_cache = {}


def _p2off(c):
    # P2 chunk slot == Y1 chunk slot (in-place, reverse processing order)
    return DOFF + CHUNK * c


def _build(a1_vals, a2_vals):
    import concourse.bass as bass
    import concourse.tile as tile
    from concourse import bacc, mybir

    f32 = mybir.dt.float32
    f32r = mybir.dt.float32r
    bf16 = mybir.dt.bfloat16
    Alu = mybir.AluOpType
    Act = mybir.ActivationFunctionType

    nc = bacc.Bacc(None, target_bir_lowering=False, debug=False, num_devices=NCORES)

    # f32r = bitwise f32; lets layer-0 matmuls consume staged x directly
    xin_d = nc.dram_tensor("xin", [BLOC, CB, T], f32r, kind="ExternalInput")
    w1f_d = nc.dram_tensor("w1f", [128, 2, 4, 128], f32r, kind="ExternalInput")
    w1t_d = nc.dram_tensor("w1t", [L, 128, 2, 4, 128], bf16, kind="ExternalInput")
    w2t_d = nc.dram_tensor("w2t", [L, 128, 4, 2, 128], bf16, kind="ExternalInput")
    vecs_d = nc.dram_tensor("vecs", [128, L, NVEC, 4], f32, kind="ExternalInput")
    b2_d = nc.dram_tensor("b2v", [128, L, 2], f32, kind="ExternalInput")
    eye_d = nc.dram_tensor("eye", [128, 128], bf16, kind="ExternalInput")
    out_d = nc.dram_tensor("out", [BLOC, CB, T], f32, kind="ExternalOutput")

    with tile.TileContext(nc) as tc:
        with (
            tc.tile_pool(name="ybufp", bufs=1) as ybufp,
            tc.tile_pool(name="constp", bufs=1) as constp,
            tc.tile_pool(name="wtmp", bufs=2) as wtmpp,
            tc.tile_pool(name="wr", bufs=2) as wrp,
            tc.tile_pool(name="xp", bufs=8) as xp,
            tc.tile_pool(name="stage", bufs=4) as stagep,
            tc.tile_pool(name="stats", bufs=1) as statsp,
            tc.tile_pool(name="vec", bufs=10) as vecp,
            # Separate PSUM rings for (A,B) vs C: sharing one FIFO ring couples
            # next-layer A-matmul slot waits to C-ACT completions, which wait
            # on xt slots released by those same A matmuls -> scheduler deadlock.
            tc.tile_pool(name="psab", bufs=6, space="PSUM") as psp,
            tc.tile_pool(name="psc", bufs=2, space="PSUM") as pscp,
            tc.tile_pool(name="dram", bufs=4, space="DRAM") as dramp,
        ):
            # persistent Y1/P2 buffers, one per channel-tile of D
            ybuf = [
                ybufp.tile([128, BLOC, YCOLS], bf16, tag=f"ybuf{ct}", name=f"ybuf{ct}")
                for ct in range(4)
            ]
            # SBUF-resident bf16 copy of x: layer-0 matmul rhs + final residual
            xres = [
                ybufp.tile([128, BLOC, T], bf16, tag=f"xres{kt}", name=f"xres{kt}")
                for kt in range(2)
            ]

            # The first-needed layer-0 inputs go to the FRONT of both DMA
            # queues: each queued DMA costs ~1.2us of sequencer issue time,
            # so consts behind them would delay the first A matmul.
            HSTG = 4 * CHUNK
            stagep_tiles = {}

            def stage_dma(b, hh, kt):
                tmp = stagep.tile([128, HSTG], f32r, tag="stage",
                                  bufs=6, name=f"stg_{b}_{hh}_{kt}")
                dma_eng = nc.sync if kt == 0 else nc.scalar
                dma_eng.dma_start(
                    tmp[:],
                    xin_d[b, 128 * kt:128 * (kt + 1), HSTG * hh:HSTG * (hh + 1)],
                )
                stagep_tiles[(b, hh, kt)] = tmp
                return tmp

            w1r0 = wtmpp.tile([128, 2, 4, 128], f32r, tag="w1r0", bufs=1)
            nc.scalar.dma_start(w1r0[:], w1f_d[:])
            stage_dma(0, 1, 0)
            stage_dma(0, 1, 1)

            # constants
            vecs_sb = constp.tile([128, L, NVEC, 4], f32)
            b2_sb = constp.tile([128, L, 2], f32)
            eye_sb = constp.tile([128, 128], bf16)
            nc.sync.dma_start(vecs_sb[:], vecs_d[:])
            nc.sync.dma_start(b2_sb[:], b2_d[:])
            nc.sync.dma_start(eye_sb[:], eye_d[:])

            # startup alignment: dummy collective issued first; the CC path
            # serializes per core, so the first stats collective starts with
            # cores already aligned instead of eating ~20us of peer skew.
            aln_in = dramp.tile([128, 1], f32, tag="alnin")
            aln_out = dramp.tile([NCORES, 128, 1], f32, tag="alnout")
            alnsb = constp.tile([128, 1], f32, tag="alnsb")
            nc.vector.memset(alnsb[:], 0.0)
            nc.scalar.dma_start(aln_in[:], alnsb[:])
            nc.gpsimd.collective_compute(
                "AllGather", Alu.bypass,
                replica_groups=[list(range(NCORES))],
                ins=[aln_in[:].opt()], outs=[aln_out[:].opt()],
            )

            # zero the causal pads (rewritten per layer with u = -t1/s1)
            zt = constp.tile([128, PAD], bf16, tag="zt")
            nc.vector.memset(zt[:], 0.0)
            ones_pad = constp.tile([128, PAD], bf16, tag="ones_pad")
            nc.vector.memset(ones_pad[:], 1.0)
            epsc = constp.tile([128, 1], f32, tag="epsc")
            nc.vector.memset(epsc[:], EPS)
            for ct in range(4):
                for b in range(BLOC):
                    nc.gpsimd.tensor_copy(ybuf[ct][:, b, 0:PAD], zt[:])

            # stage x into SBUF (reverse chunk order = consumption order of
            # layer-0 phase A).  The staged tiles are f32r so layer-0 A
            # matmuls consume them DIRECTLY (f32r streams 1 row/cycle at 512
            # cols) - no conversion copy on the critical path.  The bf16
            # xres copies (needed only for the final residual) run on the
            # otherwise-idle Pool engine.
            xstg = {}
            for b in range(BLOC):
                for hh in (1, 0):  # halves of T, 2048 cols each
                    for kt in range(2):
                        tmp = stagep_tiles.get((b, hh, kt))
                        if tmp is None:
                            tmp = stage_dma(b, hh, kt)
                        xstg[(b, hh, kt)] = tmp
                        dst = xres[kt][:, b, HSTG * hh:HSTG * (hh + 1)]
                        nc.gpsimd.tensor_copy(dst, tmp[:])

            def load_layer_weights(i):
                w1sb = None
                if i > 0:  # layer 0 uses the f32r copy w1r0
                    w1sb = wtmpp.tile([128, 2, 4, 128], bf16, tag="w1sb")
                    nc.sync.dma_start(w1sb[:], w1t_d[i])

                # diagonal dconv weight blocks: eye * wd_tap (per-partition scalar)
                diagr = wrp.tile([128, 3, 4, 128], bf16, tag="diagr")
                for j in range(3):
                    for ct in range(4):
                        nc.vector.tensor_scalar(
                            diagr[:, j, ct, :], eye_sb[:],
                            vecs_sb[:, i, V_WD0 + j, ct:ct + 1], None,
                            op0=mybir.AluOpType.mult,
                        )

                w2sb = wtmpp.tile([128, 4, 2, 128], bf16, tag="w2sb")
                nc.sync.dma_start(w2sb[:], w2t_d[i])
                return w1sb, diagr, w2sb

            def stat_sync(i, st, g_idx, be_idx, want_t=True):
                """Direct (sum, sumsq) from bn_stats triples + AllGather.

                st: [128, 4(ct), NCHUNKS, 6]; each 6 = two (count=256, mean, M2)
                triples. sum = 256*reduce(means); sumsq = reduce(M2 + 256*mean^2).
                The 4KB (sum, sumsq) vector is AllGathered (cheaper than
                AllReduce on the CC path) and summed locally on DVE.
                Returns (s4, t4, mean4, rstd4); t4 None unless want_t.
                """
                st3 = st[:].rearrange("p ct ch (h s) -> p ct (ch h) s", s=3)
                means = st3[:, :, :, 1]
                m2s = st3[:, :, :, 2]
                csb = vecp.tile([128, 8], f32, tag="csb", bufs=4)
                sums_r = vecp.tile([128, 4], f32, tag="sums_r")
                nc.vector.tensor_reduce(
                    sums_r[:], means, axis=mybir.AxisListType.X, op=Alu.add
                )
                nc.vector.tensor_scalar(
                    csb[:, 0:4], sums_r[:], float(CHUNK // 2), None, op0=Alu.mult
                )
                msq = vecp.tile([128, 4, 2 * NCHUNKS], f32, tag="msq", bufs=2)
                nc.vector.tensor_mul(msq[:], means, means)
                nc.vector.scalar_tensor_tensor(
                    msq[:], msq[:], float(CHUNK // 2), m2s,
                    op0=Alu.mult, op1=Alu.add,
                )
                nc.vector.tensor_reduce(
                    csb[:, 4:8], msq[:], axis=mybir.AxisListType.X, op=Alu.add
                )
                cin = dramp.tile([128, 8], f32, tag="cin")
                cga = dramp.tile([NCORES, 128, 8], f32, tag="cga")
                nc.sync.dma_start(cin[:], csb[:])
                nc.gpsimd.collective_compute(
                    "AllGather", Alu.bypass,
                    replica_groups=[list(range(NCORES))],
                    ins=[cin[:].opt()], outs=[cga[:].opt()],
                )
                gsb8 = vecp.tile([128, NCORES, 8], f32, tag="gsb8")
                nc.sync.dma_start(gsb8[:], cga[:].rearrange("c p s -> p c s"))
                gsb = vecp.tile([128, 8], f32, tag="gsb")
                nc.vector.tensor_reduce(
                    gsb[:], gsb8[:].rearrange("p c s -> p s c"),
                    axis=mybir.AxisListType.X, op=Alu.add,
                )

                mv8 = vecp.tile([128, 8], f32, tag="mv8")
                nc.vector.tensor_scalar(
                    mv8[:], gsb[:], 1.0 / NGLOB, None, op0=Alu.mult
                )
                mean4 = mv8[:, 0:4]
                var4 = vecp.tile([128, 4], f32, tag="var4")
                nc.vector.tensor_mul(var4[:], mean4, mean4)
                nc.vector.tensor_sub(var4[:], mv8[:, 4:8], var4[:])
                std4 = vecp.tile([128, 4], f32, tag="std4")
                nc.scalar.activation(std4[:], var4[:], Act.Sqrt, bias=epsc[:], scale=1.0)
                rstd4 = vecp.tile([128, 4], f32, tag="rstd4")
                nc.vector.reciprocal(rstd4[:], std4[:])
                s4 = vecp.tile([128, 4], f32, tag="s4")
                nc.vector.tensor_mul(s4[:], rstd4[:], vecs_sb[:, i, g_idx, :])
                t4 = None
                if want_t:
                    t4 = vecp.tile([128, 4], f32, tag="t4")
                    nc.vector.tensor_mul(t4[:], mean4, s4[:])
                    nc.vector.tensor_sub(t4[:], vecs_sb[:, i, be_idx, :], t4[:])
                return s4, t4, mean4, rstd4

            xtiles = {}

            for i in range(L):
                dil = 2 ** i
                a1i = float(a1_vals[i])
                a2i = float(a2_vals[i])
                w1sb, diagr, w2sb = load_layer_weights(i)

                # ---- Phase A: w1 + prelu1 + stats1 ----
                st1 = statsp.tile([128, 4, NCHUNKS, 6], f32, tag="st1",
                                  name=f"st1_{i}")
                for b in range(BLOC):
                    for c in REV:
                        for mt in range(4):
                            ps = psp.tile([128, CHUNK], f32, tag="ps")
                            for kt in range(2):
                                if i == 0:
                                    hh, off = divmod(CHUNK * c, HSTG)
                                    rhs = xstg[(b, hh, kt)][:, off:off + CHUNK]
                                    lhsT = w1r0[:, kt, mt, :]
                                else:
                                    rhs = xtiles[(kt, b, c)][:]
                                    lhsT = w1sb[:, kt, mt, :]
                                nc.tensor.matmul(
                                    ps[:],
                                    lhsT,
                                    rhs,
                                    start=(kt == 0), stop=(kt == 1),
                                )
                            ysl = ybuf[mt][:, b, DOFF + CHUNK * c:DOFF + CHUNK * (c + 1)]
                            nc.scalar.activation(
                                ysl, ps[:], Act.Prelu,
                                bias=vecs_sb[:, i, V_B1, mt:mt + 1], scale=1.0, alpha=a1i,
                            )
                            nc.vector.bn_stats(st1[:, mt, b * NCT + c, :], ysl)

                # dconv run-ahead: emit the first 6 tap-matmul groups (1.5
                # chunks, the full psab ring) BEFORE the sync so the PE works
                # through the collective latency. Their ACT drains are emitted
                # in the normal phase-B loop (gated on s1).
                def emit_dconv(b, c, ct):
                    ps = psp.tile([128, CHUNK], f32, tag="ps",
                                  name=f"ps_{i}_{b}_{c}_{ct}")
                    base = DOFF + CHUNK * c
                    for j in range(KTAP):
                        off = base - (2 - j) * dil
                        nc.tensor.matmul(
                            ps[:],
                            diagr[:, j, ct, :],
                            ybuf[ct][:, b, off:off + CHUNK],
                            start=(j == 0), stop=(j == KTAP - 1),
                        )
                    return ps

                def emit_dconv_psc(b, c, ct):
                    # psc ring is idle between phase C(i-1) and C(i); borrow
                    # 2 banks for extra dconv run-ahead during sync1
                    ps = pscp.tile([128, CHUNK], f32, tag="psc",
                                   name=f"psx_{i}_{b}_{c}_{ct}")
                    base = DOFF + CHUNK * c
                    for j in range(KTAP):
                        off = base - (2 - j) * dil
                        nc.tensor.matmul(
                            ps[:],
                            diagr[:, j, ct, :],
                            ybuf[ct][:, b, off:off + CHUNK],
                            start=(j == 0), stop=(j == KTAP - 1),
                        )
                    return ps

                ra_psums = {}
                for (rb, rc, rct) in [(0, NCT - 1, 0), (0, NCT - 1, 1),
                                      (0, NCT - 1, 2), (0, NCT - 1, 3),
                                      (0, NCT - 2, 0), (0, NCT - 2, 1)]:
                    ra_psums[(rb, rc, rct)] = emit_dconv(rb, rc, rct)
                for (rb, rc, rct) in [(0, NCT - 2, 2), (0, NCT - 2, 3)]:
                    ra_psums[(rb, rc, rct)] = emit_dconv_psc(rb, rc, rct)

                # ---- sync1 ----
                s1, t1, mean1, rstd1 = stat_sync(i, st1, V_G1, V_BE1, want_t=True)
                # uniform bias via host-folded constants: b2a = K_a - (mean*rstd)*W_a
                mr1 = vecp.tile([128, 4], f32, tag="mr1")
                nc.vector.tensor_mul(mr1[:], mean1, rstd1[:])
                b2a4 = vecp.tile([128, 4], f32, tag="b2a4")
                nc.vector.tensor_mul(b2a4[:], mr1[:], vecs_sb[:, i, V_WA, :])
                nc.vector.tensor_sub(b2a4[:], vecs_sb[:, i, V_KA, :], b2a4[:])
                # Causal-pad self-correction: writing u = -t1/s1 into the pad
                # columns makes every pad tap contribute exactly the missing
                # -t1*w_j bias term, so ONE uniform apply (scale=s1, bias=
                # t1*wsum+bd) is correct for every chunk - no boundary-strip
                # fixups.  (Assumes s1 != 0, i.e. BN weight g1 has no zeros.)
                rec1 = vecp.tile([128, 4], f32, tag="rec1")
                nc.vector.reciprocal(rec1[:], s1[:])
                u4 = vecp.tile([128, 4], f32, tag="u4")
                nc.vector.tensor_mul(u4[:], t1[:], rec1[:])
                nc.vector.tensor_scalar(u4[:], u4[:], -1.0, None, op0=Alu.mult)
                for ct in range(4):
                    for b in range(BLOC):
                        nc.vector.tensor_scalar(
                            ybuf[ct][:, b, 0:PAD], ones_pad[:],
                            u4[:, ct:ct + 1], None, op0=Alu.mult,
                        )

                # ---- Phase B: dconv (PE diag matmuls) + bn1-apply + prelu2 + stats2 ----
                st2 = statsp.tile([128, 4, NCHUNKS, 6], f32, tag="st2",
                                  name=f"st2_{i}")
                for b in range(BLOC):
                    for c in REV:
                        for ct in range(4):
                            if (b, c, ct) in ra_psums:
                                ps = ra_psums[(b, c, ct)]
                            else:
                                ps = emit_dconv(b, c, ct)
                            po = _p2off(c)
                            p2sl = ybuf[ct][:, b, po:po + CHUNK]
                            nc.scalar.activation(
                                p2sl, ps[:], Act.Prelu,
                                bias=b2a4[:, ct:ct + 1], scale=s1[:, ct:ct + 1], alpha=a2i,
                            )
                            nc.vector.bn_stats(st2[:, ct, b * NCT + c, :], p2sl)

                # ---- sync2 + weight folding ----
                s2, t2, _, _ = stat_sync(i, st2, V_G2, V_BE2)
                w2r = wrp.tile([128, 4, 2, 128], bf16, tag="w2r")
                # r2 = t2 / s2  (so bias3 = W2' @ r2 + b2); 2 identical cols
                rec4 = vecp.tile([128, 4], f32, tag="rec4")
                nc.vector.reciprocal(rec4[:], s2[:])
                r24 = vecp.tile([128, 4, 2], bf16, tag="r24")
                nc.vector.tensor_mul(r24[:, :, 0], t2[:], rec4[:])
                nc.vector.tensor_mul(r24[:, :, 1], t2[:], rec4[:])
                for kt in range(4):
                    nc.vector.tensor_scalar(
                        w2r[:, kt, :, :], w2sb[:, kt, :, :], s2[:, kt:kt + 1], None,
                        op0=Alu.mult,
                    )
                psb = pscp.tile([128, 2, 2], f32, tag="psc", name="psb")
                for mt in range(2):
                    for kt in range(4):
                        nc.tensor.matmul(
                            psb[:, mt, :],
                            w2r[:, kt, mt, :],
                            r24[:, kt, :],
                            start=(kt == 0), stop=(kt == 3),
                        )
                bias3 = []
                for mt in range(2):
                    b3 = vecp.tile([128, 1], f32, tag="b3")
                    nc.scalar.activation(
                        b3[:], psb[:, mt, 0:1], Act.Identity,
                        bias=b2_sb[:, i, mt:mt + 1], scale=1.0,
                    )
                    bias3.append(b3)

                # ---- Phase C: w2 + bias (+ residual on last layer) ----
                # Drains alternate DVE/ACT so neither engine is the wall.
                xtiles_next = {}
                obufs = {}
                for b in range(BLOC):
                    for c in REV:
                        po = _p2off(c)
                        for mt in range(2):
                            ps = pscp.tile([128, CHUNK], f32, tag="psc")
                            for kt in range(4):
                                nc.tensor.matmul(
                                    ps[:],
                                    w2r[:, kt, mt, :],
                                    ybuf[kt][:, b, po:po + CHUNK],
                                    start=(kt == 0), stop=(kt == 3),
                                )
                            if i < L - 1:
                                # alternate drain engines: DVE and ACT each
                                # take half, so neither becomes the wall of
                                # the C+A(i+1) window (PE is)
                                xt = xp.tile([128, CHUNK], bf16, tag="xt")
                                if (c + mt) % 2 == 0:
                                    nc.vector.tensor_scalar(
                                        xt[:], ps[:], bias3[mt][:], None, op0=Alu.add,
                                    )
                                else:
                                    nc.scalar.activation(
                                        xt[:], ps[:], Act.Identity,
                                        bias=bias3[mt][:], scale=1.0,
                                    )
                                xtiles_next[(mt, b, c)] = xt
                            else:
                                # drain into [128,1024] pair tiles; one DMA
                                # per completed pair halves the SP-sequencer
                                # issue load that otherwise bounds the tail
                                pr = obufs.get((b, mt, c // 2))
                                if pr is None:
                                    pr = stagep.tile([128, 2 * CHUNK], f32,
                                                     tag="ro", bufs=4)
                                    obufs[(b, mt, c // 2)] = pr
                                osl = pr[:, CHUNK * (c % 2):CHUNK * (c % 2 + 1)]
                                nc.vector.scalar_tensor_tensor(
                                    osl, ps[:], bias3[mt][:],
                                    xres[mt][:, b, CHUNK * c:CHUNK * (c + 1)],
                                    op0=Alu.add, op1=Alu.add,
                                )
                                if c % 2 == 0:
                                    nc.sync.dma_start(
                                        out_d[b, 128 * mt:128 * (mt + 1),
                                              2 * CHUNK * (c // 2):2 * CHUNK * (c // 2 + 1)],
                                        pr[:],
                                    )
                xtiles = xtiles_next

    nc.compile()
    return nc


def _prep_inputs(x, w1, b1, a1, g1, be1, wd, bd, a2, g2, be2, w2, b2):
    """Host-side packing. All weights binarized via sign()."""
    import ml_dtypes

    bf16 = ml_dtypes.bfloat16
    w1b = np.sign(w1[..., 0]).astype(np.float32)  # [L, D, CB]
    wdb = np.sign(wd[..., 0, :]) if wd.ndim == 4 else np.sign(wd[:, :, 0, :])
    wdb = wdb.astype(np.float32)  # [L, D, K]
    w2b = np.sign(w2[..., 0]).astype(np.float32)  # [L, CB, D]

    w1t = np.empty((L, 128, 2, 4, 128), np.float32)
    w2t = np.empty((L, 128, 4, 2, 128), np.float32)
    for i in range(L):
        # lhsT[k, m] = w[m_global, k_global]
        for kt in range(2):
            for mt in range(4):
                blk = w1b[i, 128 * mt:128 * (mt + 1), 128 * kt:128 * (kt + 1)]
                w1t[i, :, kt, mt, :] = blk.T
        for kt in range(4):
            for mt in range(2):
                blk = w2b[i, 128 * mt:128 * (mt + 1), 128 * kt:128 * (kt + 1)]
                w2t[i, :, kt, mt, :] = blk.T

    wsa = wdb.sum(-1)  # [L, D]
    ws12 = wdb[:, :, 1] + wdb[:, :, 2]
    ws2 = wdb[:, :, 2]
    # host-folded bias2-variant constants: b2x = K_x - (mean*rstd)*W_x
    ka, k12, k2 = (be1 * wsa + bd, be1 * ws12 + bd, be1 * ws2 + bd)
    wa, w12, w2c = (g1 * wsa, g1 * ws12, g1 * ws2)
    vec_list = [b1, bd, g1, be1, g2, be2, ka, k12, k2,
                wdb[:, :, 0], wdb[:, :, 1], wdb[:, :, 2], wa, w12, w2c]
    vecs = np.empty((128, L, NVEC, 4), np.float32)
    for v, arr in enumerate(vec_list):
        # arr [L, D] -> [128(p), L, ct]
        vecs[:, :, v, :] = arr.reshape(L, 4, 128).transpose(2, 0, 1)
    b2v = b2.reshape(L, 2, 128).transpose(2, 0, 1).astype(np.float32)  # [128, L, 2]
    eye = np.eye(128, dtype=np.float32)
    return (w1t.astype(bf16), w1t[0].astype(np.float32), w2t.astype(bf16),
            vecs, b2v, eye.astype(bf16))


def kernel(**inputs):
    from concourse.bass_utils import run_bass_kernel_spmd

    inputs = {k: np.asarray(v, dtype=np.float32) for k, v in inputs.items()}
    x = inputs["x"]
    w1t, w1f, w2t, vecs, b2v, eye = _prep_inputs(**inputs)

    key = "nc"
    if key not in _cache:
        _cache[key] = _build(inputs["a1"], inputs["a2"])
    nc = _cache[key]

    in_maps = []
    for i in range(NCORES):
        in_maps.append({
            "xin": np.ascontiguousarray(x[BLOC * i:BLOC * (i + 1)]),
            "w1t": w1t, "w1f": w1f, "w2t": w2t, "vecs": vecs, "b2v": b2v,
            "eye": eye,
        })
    import os
    trace = bool(int(os.environ.get("BASS_KERNEL_TRACE", "0")))
    res = run_bass_kernel_spmd(
        nc, in_maps, core_ids=list(range(NCORES)), trace=trace,
    )
    _cache["last_results"] = res
    out = np.empty((B, CB, T), np.float32)
    for i in range(NCORES):
        out[BLOC * i:BLOC * (i + 1)] = res.results[i]["out"]
    return out
